# revision 11
# baseline (speedup 1.0000x reference)
"""Trainium2 Bass kernel for nn_FNDE (FNO neural-ODE).

Mathematical collapse (validated to ~5e-7 rel err vs the f32 jax reference):
Each Fourier layer's RK4 integrates dz/dt = f(z) where f (the FNO spectral
conv) is affine, and *linear per rfft2 mode*: retained modes evolve through a
CxC complex matrix, non-retained modes are untouched, the bias only feeds mode
(0,0).  Lift (1->C) and projection (C->64->1) are pointwise linear.  Hence the
whole network is a per-mode complex scalar acting on rfft2(z):

    out[b] = s_base * z[b] + irfft2(S' * rfft2(z[b]))  + c_total

with S' supported on 33 x-rows (kx in [0..16] u [112..127]) x 16 y-cols.
The ky=0 column needs care: irfft2's implicit Hermitian projection couples the
two retained row blocks (top via W1, bottom via conj(W2)) each evaluation.

The device kernel evaluates the restricted DFT chain as TensorEngine matmuls
per sample (data-parallel: 2 samples per core across 8 cores):
    PT  = z.T @ FxC                  (x-DFT, re/im fused, output transposed)
    QQT = CyS.T @ PT                 (y-DFT cos/sin blocks)
    Q   = combine(QQT)               (complex assembly, transposed layout)
    W'  = S' (.) Q                   (elementwise spectral multiplier)
    V   = [W'r;W'i] via L1.T@CEy + L2.T@SEy   (y-inverse)
    O   = ICIS.T @ V + s_base * z    (x-inverse + identity path, PSUM accum)
    out = O + c_total
"""

import numpy as np

B, C, D, M, L = 16, 64, 128, 16, 3
NCORES = 8
SPC = B // NCORES  # samples per core
KXS = np.concatenate([np.arange(17), np.arange(112, 128)])  # 33 retained rows
NR = len(KXS)  # 33


# ----------------------------------------------------------------------------
# host-side weight folding (numpy, float64)
# ----------------------------------------------------------------------------

def _rk4(f, x, ts):
    for i in range(len(ts) - 1):
        h = ts[i + 1] - ts[i]
        k1 = f(x)
        k2 = f(x + 0.5 * h * k1)
        k3 = f(x + 0.5 * h * k2)
        k4 = f(x + h * k3)
        x = x + (h / 6.0) * (k1 + 2 * k2 + 2 * k3 + k4)
    return x


def _fold_weights(inputs):
    lift_w = np.asarray(inputs["lift_w"], np.float64)[:, 0]      # [C]
    lift_b = np.asarray(inputs["lift_b"], np.float64)            # [C]
    w1 = np.asarray(inputs["spec_w1"], np.float64)               # [L,C,C,M,M,2]
    w2 = np.asarray(inputs["spec_w2"], np.float64)
    fl_bias = np.asarray(inputs["fl_bias"], np.float64)          # [L,C]
    p1_w = np.asarray(inputs["p1_w"], np.float64)
    p1_b = np.asarray(inputs["p1_b"], np.float64)
    p2_w = np.asarray(inputs["p2_w"], np.float64)
    p2_b = np.asarray(inputs["p2_b"], np.float64)
    ts = np.asarray(inputs["samp_ts"], np.float64)

    q = (p2_w @ p1_w)[0]                                         # [C]
    c_proj = float((p2_w @ p1_b + p2_b)[0])
    s_base = float(q @ lift_w)

    w1c = w1[..., 0] + 1j * w1[..., 1]                           # [L,C(i),C(o),M,M]
    w2c = w2[..., 0] + 1j * w2[..., 1]
    # einsum "bixy,ioxy->boxy": out_o = sum_i v_i W[i,o]  =>  generator = W^T
    G1 = np.transpose(w1c, (0, 4, 3, 2, 1))                      # [L,ky,kx,o,i]
    G2 = np.transpose(w2c, (0, 4, 3, 2, 1))                      # [L,ky,r,o,i] rows 112+r

    qc = q.astype(np.complex128)

    def chain(G_stack):
        # propagate lift_w through L layers of 4-step RK4 (linear, no bias),
        # then project with q -> per-mode scalar
        x = np.broadcast_to(lift_w, G_stack.shape[1:-2] + (C,)).astype(np.complex128)
        for layer in range(L):
            Gl = G_stack[layer]
            x = _rk4(lambda v: np.einsum("...ij,...j->...i", Gl, v), x, ts)
        return np.einsum("c,...c->...", qc, x)

    s_top = chain(G1[:, 1:])                                     # [15(ky=1..15),16(kx)]
    s_bot = chain(G2[:, 1:])                                     # [15,16(r)]

    # ky = 0 column: Hermitian projection couples the blocks. Independent
    # coords m in [0,16]; generators per layer:
    G0 = np.zeros((L, 17, C, C), np.complex128)
    for layer in range(L):
        G0[layer, 0] = np.real(G1[layer, 0, 0])
        for m in range(1, 16):
            G0[layer, m] = 0.5 * (G1[layer, 0, m] + np.conj(G2[layer, 0, 16 - m]))
        G0[layer, 16] = 0.5 * np.conj(G2[layer, 0, 0])
    s_col0 = chain(G0)                                           # [17]

    # affine offset at mode (0,0): propagate lift_b*D^2 with layer biases
    y = (lift_b * D * D).astype(np.complex128)
    for layer in range(L):
        Gl = G0[layer, 0]
        bl = (fl_bias[layer] * D * D).astype(np.complex128)
        y = _rk4(lambda v: Gl @ v + bl, y, ts)
    t_const = float(np.real(qc @ y))
    c_total = c_proj + t_const / (D * D)

    # assemble restricted multiplier S' = S - s_base on [33 rows, 16 cols]
    Sp = np.full((NR, 16), s_base, np.complex128)
    Sp[0:16, 1:16] = s_top.T                                     # [kx, ky]
    Sp[17:33, 1:16] = s_bot.T
    Sp[0:17, 0] = s_col0
    for r in range(16):                                          # stored bottom of ky=0
        Sp[17 + r, 0] = np.conj(s_col0[16 - r])
    Sp = Sp - s_base

    # ------------------------------------------------------------------
    # constant matrices for the device matmul chain (float32)
    # ------------------------------------------------------------------
    xg = np.arange(D, dtype=np.float64)
    th_x = 2.0 * np.pi * np.outer(xg, KXS) / D                   # [x, r]
    fxc = np.concatenate([np.cos(th_x), -np.sin(th_x)], axis=1)  # [128, 66]

    ky = np.arange(16, dtype=np.float64)
    th_y = 2.0 * np.pi * np.outer(xg, ky) / D                    # [y, ky]
    cys = np.concatenate([np.cos(th_y), np.sin(th_y)], axis=1)   # [128, 32]

    cc = np.where(ky == 0, 1.0, 2.0) / D                         # [16]
    cey = cc[:, None] * np.cos(th_y.T)                           # [16, 128]
    sey = cc[:, None] * np.sin(th_y.T)

    icis = np.concatenate([np.cos(th_x.T), -np.sin(th_x.T)], axis=0) / D  # [66,128]
    si = s_base * np.eye(D)

    srt = Sp.real.T.astype(np.float32)                           # [16, 33]
    sit = Sp.imag.T.astype(np.float32)

    return dict(
        cbias=np.full((D, 1), c_total, np.float32),
        fxc=fxc.astype(np.float32),
        cys=cys.astype(np.float32),
        cey=cey.astype(np.float32),
        sey=sey.astype(np.float32),
        icis=icis.astype(np.float32),
        si=si.astype(np.float32),
        srt4=np.tile(srt, (1, 2 * SPC)).reshape(16, SPC, 2, NR).astype(np.float32),
        sit4=np.tile(sit, (1, 2 * SPC)).reshape(16, SPC, 2, NR).astype(np.float32),
        c_total=np.float32(c_total),
    )


# ----------------------------------------------------------------------------
# numpy simulation of the exact device chain (for validation / fallback)
# ----------------------------------------------------------------------------

def _device_sim(z2, k):
    """z2: [SPC,128,128] f32; k: folded consts. Mirrors the Bass kernel."""
    out = np.empty_like(z2)
    for s in range(SPC):
        z = z2[s]
        pt = z.T @ k["fxc"]                                      # [y, 66]
        qc = k["cys"][:, 0:16].T @ pt                            # [16, 66]
        qs = k["cys"][:, 16:32].T @ pt
        qr = qc[:, 0:33] + qs[:, 33:66]                          # [ky, r]
        qi = qc[:, 33:66] - qs[:, 0:33]
        srt = k["srt4"][:, 0, 0, :]
        sit = k["sit4"][:, 0, 0, :]
        wr = srt * qr - sit * qi
        wi = srt * qi + sit * qr
        l1 = np.concatenate([wr, wi], axis=1)                    # [16, 66]
        l2 = np.concatenate([-wi, wr], axis=1)
        v = l1.T @ k["cey"] + l2.T @ k["sey"]                    # [66, 128]
        o = k["icis"].T @ v + k["si"].T @ z                      # [128, 128]
        out[s] = o + k["c_total"]
    return out


def _shard_inputs(z, consts):
    in_maps = []
    for i in range(NCORES):
        m = {"zc": np.ascontiguousarray(z[i * SPC:(i + 1) * SPC, 0])}
        for name in ("fxc", "cys", "cey", "sey", "icis", "si", "srt4", "sit4",
                     "cbias"):
            m[name] = consts[name]
        in_maps.append(m)
    return in_maps


# ----------------------------------------------------------------------------
# Bass kernel
# ----------------------------------------------------------------------------

_NC_CACHE = {}
LAST_RESULT = None  # BassKernelResults of the most recent device run


def _build_nc():
    import concourse.bacc as bacc
    import concourse.mybir as mybir
    import concourse.tile as tile

    f32 = mybir.dt.float32
    nc = bacc.Bacc("TRN2", target_bir_lowering=False, debug=False,
                   num_devices=NCORES)
    zc = nc.dram_tensor("zc", [SPC, D, D], f32, kind="ExternalInput")
    fxc = nc.dram_tensor("fxc", [D, 2 * NR], f32, kind="ExternalInput")
    cys = nc.dram_tensor("cys", [D, 32], f32, kind="ExternalInput")
    cey = nc.dram_tensor("cey", [16, D], f32, kind="ExternalInput")
    sey = nc.dram_tensor("sey", [16, D], f32, kind="ExternalInput")
    icis = nc.dram_tensor("icis", [2 * NR, D], f32, kind="ExternalInput")
    si = nc.dram_tensor("si", [D, D], f32, kind="ExternalInput")
    srt4 = nc.dram_tensor("srt4", [16, SPC, 2, NR], f32, kind="ExternalInput")
    sit4 = nc.dram_tensor("sit4", [16, SPC, 2, NR], f32, kind="ExternalInput")
    cbias = nc.dram_tensor("cbias", [D, 1], f32, kind="ExternalInput")
    outc = nc.dram_tensor("outc", [SPC, D, D], f32, kind="ExternalOutput")

    W = 2 * NR  # 66

    with tile.TileContext(nc) as tc:
        with (
            tc.tile_pool(name="const", bufs=1) as cpool,
            tc.tile_pool(name="work", bufs=2) as wpool,
            tc.tile_pool(name="psum", bufs=2, space="PSUM") as ppool,
        ):
            t_fxc = cpool.tile([D, W], f32)
            t_cys = cpool.tile([D, 32], f32)
            t_cey = cpool.tile([16, D], f32)
            t_sey = cpool.tile([16, D], f32)
            t_icis = cpool.tile([W, D], f32)
            t_si = cpool.tile([D, D], f32)
            t_srt4 = cpool.tile([16, SPC, 2, NR], f32)
            t_sit4 = cpool.tile([16, SPC, 2, NR], f32)
            t_cbias = cpool.tile([D, 1], f32)
            nc.sync.dma_start(t_cbias[:], cbias[:])
            nc.sync.dma_start(t_fxc[:], fxc[:])
            nc.sync.dma_start(t_cys[:], cys[:])
            nc.sync.dma_start(t_cey[:], cey[:])
            nc.sync.dma_start(t_sey[:], sey[:])
            nc.sync.dma_start(t_icis[:], icis[:])
            nc.sync.dma_start(t_si[:], si[:])
            nc.sync.dma_start(t_srt4[:], srt4[:])
            nc.sync.dma_start(t_sit4[:], sit4[:])

            t_z = wpool.tile([D, SPC, D], f32)
            nc.sync.dma_start(t_z[:], zc.rearrange("s x y -> x s y"))

            # x-forward DFT (output already transposed): PT_s = z_s.T @ FxC
            t_ptb = wpool.tile([D, SPC, W], f32)
            for s in range(SPC):
                pt_ps = ppool.tile([D, W], f32)
                nc.tensor.matmul(pt_ps[:], t_z[:, s, :], t_fxc[:],
                                 start=True, stop=True)
                nc.scalar.copy(t_ptb[:, s, :], pt_ps[:])

            # y-forward DFT, cos and sin blocks both on partitions 0:16
            # (two-input DVE ops require equal base partitions)
            qq_ps = ppool.tile([16, 2, SPC, W], f32)
            nc.tensor.matmul(qq_ps[:, 0, :, :], t_cys[:, 0:16], t_ptb[:],
                             start=True, stop=True)
            nc.tensor.matmul(qq_ps[:, 1, :, :], t_cys[:, 16:32], t_ptb[:],
                             start=True, stop=True)

            # complex assembly (transposed layout [ky, r]); TensorTensor can
            # read only one PSUM input, so stage qq through SBUF first
            t_qq = wpool.tile([16, 2, SPC, W], f32)
            nc.vector.tensor_copy(t_qq[:], qq_ps[:])
            t_qt = wpool.tile([16, SPC, 2, NR], f32)
            for s in range(SPC):
                nc.vector.tensor_add(t_qt[:, s, 0, :],
                                     t_qq[:, 0, s, 0:NR], t_qq[:, 1, s, NR:W])
                nc.vector.tensor_sub(t_qt[:, s, 1, :],
                                     t_qq[:, 0, s, NR:W], t_qq[:, 1, s, 0:NR])

            # spectral multiplier
            t_m1 = wpool.tile([16, SPC, 2, NR], f32)
            t_m2 = wpool.tile([16, SPC, 2, NR], f32)
            nc.vector.tensor_mul(t_m1[:], t_srt4[:], t_qt[:])
            nc.vector.tensor_mul(t_m2[:], t_sit4[:], t_qt[:])

            t_l1 = wpool.tile([16, SPC, 2, NR], f32)
            t_l2 = wpool.tile([16, SPC, 2, NR], f32)
            for s in range(SPC):
                # W'r = Sr*Qr - Si*Qi ; W'i = Sr*Qi + Si*Qr
                nc.vector.tensor_sub(t_l1[:, s, 0, :], t_m1[:, s, 0, :], t_m2[:, s, 1, :])
                nc.vector.tensor_add(t_l1[:, s, 1, :], t_m1[:, s, 1, :], t_m2[:, s, 0, :])
                nc.scalar.mul(t_l2[:, s, 0, :], t_l1[:, s, 1, :], -1.0)
                nc.scalar.copy(t_l2[:, s, 1, :], t_l1[:, s, 0, :])

            t_out = wpool.tile([D, SPC, D], f32)
            for s in range(SPC):
                # y-inverse: V = [Vr; Vi] = L1.T @ CEy + L2.T @ SEy
                v_ps = ppool.tile([W, D], f32)
                nc.tensor.matmul(v_ps[:], t_l1[:, s, :, :], t_cey[:],
                                 start=True, stop=False)
                nc.tensor.matmul(v_ps[:], t_l2[:, s, :, :], t_sey[:],
                                 start=False, stop=True)
                t_v = wpool.tile([W, D], f32)
                nc.scalar.copy(t_v[:], v_ps[:])

                # x-inverse + s_base * z
                o_ps = ppool.tile([D, D], f32)
                nc.tensor.matmul(o_ps[:], t_icis[:], t_v[:], start=True, stop=False)
                nc.tensor.matmul(o_ps[:], t_si[:], t_z[:, s, :], start=False, stop=True)
                nc.scalar.add(t_out[:, s, :], o_ps[:], t_cbias[:, 0:1])

            nc.sync.dma_start(outc.rearrange("s x y -> x s y"), t_out[:])

    nc.finalize()
    return nc


def _run_device(z, consts):
    global LAST_RESULT
    from concourse.bass_utils import run_bass_kernel_spmd

    if "nc" not in _NC_CACHE:
        _NC_CACHE["nc"] = _build_nc()
    nc = _NC_CACHE["nc"]
    in_maps = _shard_inputs(z, consts)
    res = run_bass_kernel_spmd(nc, in_maps, core_ids=list(range(NCORES)))
    LAST_RESULT = res
    out = np.empty((B, 1, D, D), np.float32)
    for i in range(NCORES):
        out[i * SPC:(i + 1) * SPC, 0] = res.results[i]["outc"]
    return out


def kernel(z, lift_w, lift_b, spec_w1, spec_w2, fl_bias, p1_w, p1_b, p2_w, p2_b,
           samp_ts):
    inputs = dict(z=z, lift_w=lift_w, lift_b=lift_b, spec_w1=spec_w1,
                  spec_w2=spec_w2, fl_bias=fl_bias, p1_w=p1_w, p1_b=p1_b,
                  p2_w=p2_w, p2_b=p2_b, samp_ts=samp_ts)
    consts = _fold_weights(inputs)
    z = np.asarray(z, np.float32)
    return _run_device(z, consts)


def kernel_numpy(z, **kw):
    """Pure-numpy path running the same folded math (validation only)."""
    inputs = dict(z=z, **kw)
    consts = _fold_weights(inputs)
    z = np.asarray(z, np.float32)
    out = np.empty((B, 1, D, D), np.float32)
    for i in range(NCORES):
        out[i * SPC:(i + 1) * SPC, 0] = _device_sim(z[i * SPC:(i + 1) * SPC, 0], consts)
    return out


# revision 14
# speedup vs baseline: 1.2721x; 1.2721x over previous
"""Trainium2 Bass kernel for nn_FNDE (FNO neural-ODE).

Mathematical collapse (validated to ~5e-7 rel err vs the f32 jax reference):
Each Fourier layer's RK4 integrates dz/dt = f(z) where f (the FNO spectral
conv) is affine, and *linear per rfft2 mode*: retained modes evolve through a
CxC complex matrix, non-retained modes are untouched, the bias only feeds mode
(0,0).  Lift (1->C) and projection (C->64->1) are pointwise linear.  Hence the
whole network is a per-mode complex scalar acting on rfft2(z):

    out[b] = s_base * z[b] + irfft2(S' * rfft2(z[b]))  + c_total

with S' supported on 33 x-rows (kx in [0..16] u [112..127]) x 16 y-cols.
The ky=0 column needs care: irfft2's implicit Hermitian projection couples the
two retained row blocks (top via W1, bottom via conj(W2)) each evaluation.

The device kernel evaluates the restricted DFT chain as TensorEngine matmuls
per sample (data-parallel: 2 samples per core across 8 cores):
    PT  = z.T @ FxC                  (x-DFT, re/im fused, output transposed)
    QQT = CyS.T @ PT                 (y-DFT cos/sin blocks)
    Q   = combine(QQT)               (complex assembly, transposed layout)
    W'  = S' (.) Q                   (elementwise spectral multiplier)
    V   = [W'r;W'i] via L1.T@CEy + L2.T@SEy   (y-inverse)
    O   = ICIS.T @ V + s_base * z    (x-inverse + identity path, PSUM accum)
    out = O + c_total
"""

import numpy as np

B, C, D, M, L = 16, 64, 128, 16, 3
NCORES = 8
SPC = B // NCORES  # samples per core
KXS = np.concatenate([np.arange(17), np.arange(112, 128)])  # 33 retained rows
NR = len(KXS)  # 33


# ----------------------------------------------------------------------------
# host-side weight folding (numpy, float64)
# ----------------------------------------------------------------------------

def _rk4(f, x, ts):
    for i in range(len(ts) - 1):
        h = ts[i + 1] - ts[i]
        k1 = f(x)
        k2 = f(x + 0.5 * h * k1)
        k3 = f(x + 0.5 * h * k2)
        k4 = f(x + h * k3)
        x = x + (h / 6.0) * (k1 + 2 * k2 + 2 * k3 + k4)
    return x


def _fold_weights(inputs):
    lift_w = np.asarray(inputs["lift_w"], np.float64)[:, 0]      # [C]
    lift_b = np.asarray(inputs["lift_b"], np.float64)            # [C]
    w1 = np.asarray(inputs["spec_w1"], np.float64)               # [L,C,C,M,M,2]
    w2 = np.asarray(inputs["spec_w2"], np.float64)
    fl_bias = np.asarray(inputs["fl_bias"], np.float64)          # [L,C]
    p1_w = np.asarray(inputs["p1_w"], np.float64)
    p1_b = np.asarray(inputs["p1_b"], np.float64)
    p2_w = np.asarray(inputs["p2_w"], np.float64)
    p2_b = np.asarray(inputs["p2_b"], np.float64)
    ts = np.asarray(inputs["samp_ts"], np.float64)

    q = (p2_w @ p1_w)[0]                                         # [C]
    c_proj = float((p2_w @ p1_b + p2_b)[0])
    s_base = float(q @ lift_w)

    w1c = w1[..., 0] + 1j * w1[..., 1]                           # [L,C(i),C(o),M,M]
    w2c = w2[..., 0] + 1j * w2[..., 1]
    # einsum "bixy,ioxy->boxy": out_o = sum_i v_i W[i,o]  =>  generator = W^T
    G1 = np.transpose(w1c, (0, 4, 3, 2, 1))                      # [L,ky,kx,o,i]
    G2 = np.transpose(w2c, (0, 4, 3, 2, 1))                      # [L,ky,r,o,i] rows 112+r

    qc = q.astype(np.complex128)

    def chain(G_stack):
        # propagate lift_w through L layers of 4-step RK4 (linear, no bias),
        # then project with q -> per-mode scalar
        x = np.broadcast_to(lift_w, G_stack.shape[1:-2] + (C,)).astype(np.complex128)
        for layer in range(L):
            Gl = G_stack[layer]
            x = _rk4(lambda v: np.einsum("...ij,...j->...i", Gl, v), x, ts)
        return np.einsum("c,...c->...", qc, x)

    s_top = chain(G1[:, 1:])                                     # [15(ky=1..15),16(kx)]
    s_bot = chain(G2[:, 1:])                                     # [15,16(r)]

    # ky = 0 column: Hermitian projection couples the blocks. Independent
    # coords m in [0,16]; generators per layer:
    G0 = np.zeros((L, 17, C, C), np.complex128)
    for layer in range(L):
        G0[layer, 0] = np.real(G1[layer, 0, 0])
        for m in range(1, 16):
            G0[layer, m] = 0.5 * (G1[layer, 0, m] + np.conj(G2[layer, 0, 16 - m]))
        G0[layer, 16] = 0.5 * np.conj(G2[layer, 0, 0])
    s_col0 = chain(G0)                                           # [17]

    # affine offset at mode (0,0): propagate lift_b*D^2 with layer biases
    y = (lift_b * D * D).astype(np.complex128)
    for layer in range(L):
        Gl = G0[layer, 0]
        bl = (fl_bias[layer] * D * D).astype(np.complex128)
        y = _rk4(lambda v: Gl @ v + bl, y, ts)
    t_const = float(np.real(qc @ y))
    c_total = c_proj + t_const / (D * D)

    # assemble restricted multiplier S' = S - s_base on [33 rows, 16 cols]
    Sp = np.full((NR, 16), s_base, np.complex128)
    Sp[0:16, 1:16] = s_top.T                                     # [kx, ky]
    Sp[17:33, 1:16] = s_bot.T
    Sp[0:17, 0] = s_col0
    for r in range(16):                                          # stored bottom of ky=0
        Sp[17 + r, 0] = np.conj(s_col0[16 - r])
    Sp = Sp - s_base

    # ------------------------------------------------------------------
    # constant matrices for the device matmul chain (float32)
    # ------------------------------------------------------------------
    xg = np.arange(D, dtype=np.float64)
    th_x = 2.0 * np.pi * np.outer(xg, KXS) / D                   # [x, r]
    fxc = np.concatenate([np.cos(th_x), -np.sin(th_x)], axis=1)  # [128, 66]

    ky = np.arange(16, dtype=np.float64)
    th_y = 2.0 * np.pi * np.outer(xg, ky) / D                    # [y, ky]
    cys = np.concatenate([np.cos(th_y), np.sin(th_y)], axis=1)   # [128, 32]

    cc = np.where(ky == 0, 1.0, 2.0) / D                         # [16]
    cey = cc[:, None] * np.cos(th_y.T)                           # [16, 128]
    sey = cc[:, None] * np.sin(th_y.T)

    icis = np.concatenate([np.cos(th_x.T), -np.sin(th_x.T)], axis=0) / D  # [66,128]

    srt = Sp.real.T.astype(np.float32)                           # [16, 33]
    sit = Sp.imag.T.astype(np.float32)

    return dict(
        fxc=fxc.astype(np.float32),
        cys=cys.astype(np.float32),
        cey=cey.astype(np.float32),
        sey=sey.astype(np.float32),
        icis=icis.astype(np.float32),
        srt4=np.tile(srt, (1, 2 * SPC)).astype(np.float32),      # [16, 132]
        sit4=np.tile(sit, (1, 2 * SPC)).astype(np.float32),
        s_base=np.float32(s_base),
        c_total=np.float32(c_total),
    )


# blob layouts shared by host packing and the device kernel
AW = 99            # blob A: [128, AW]
A_FXC, A_CYC, A_CYS, A_SB = 0, 66, 82, 98
BW = 1032          # blob B: [66, BW]
B_ICIS, B_CEY, B_SEY, B_SRT, B_SIT, B_CROW, B_ONES = 0, 128, 256, 384, 516, 648, 776


def _pack_blobs(k):
    ba = np.zeros((D, AW), np.float32)
    ba[:, A_FXC:A_FXC + 2 * NR] = k["fxc"]
    ba[:, A_CYC:A_CYC + 32] = k["cys"]
    ba[:, A_SB] = k["s_base"]
    bb = np.zeros((2 * NR, BW), np.float32)
    bb[:, B_ICIS:B_ICIS + D] = k["icis"]
    bb[0:16, B_CEY:B_CEY + D] = k["cey"]
    bb[0:16, B_SEY:B_SEY + D] = k["sey"]
    bb[0:16, B_SRT:B_SRT + 2 * NR * SPC] = k["srt4"]
    bb[0:16, B_SIT:B_SIT + 2 * NR * SPC] = k["sit4"]
    bb[0, B_CROW:B_CROW + D] = k["c_total"]
    bb[0, B_ONES:B_ONES + SPC * D] = 1.0
    return ba, bb


# ----------------------------------------------------------------------------
# numpy simulation of the exact device chain (for validation / fallback)
# ----------------------------------------------------------------------------

def _device_sim(z2, k):
    """z2: [SPC,128,128] f32; k: folded consts. Mirrors the Bass kernel."""
    out = np.empty_like(z2)
    for s in range(SPC):
        z = z2[s]
        pt = z.T @ k["fxc"]                                      # [y, 66]
        qc = k["cys"][:, 0:16].T @ pt                            # [16, 66]
        qs = k["cys"][:, 16:32].T @ pt
        qr = qc[:, 0:33] + qs[:, 33:66]                          # [ky, r]
        qi = qc[:, 33:66] - qs[:, 0:33]
        srt = k["srt4"][:, 0:NR]
        sit = k["sit4"][:, 0:NR]
        wr = srt * qr - sit * qi
        wi = srt * qi + sit * qr
        l1 = np.concatenate([wr, wi], axis=1)                    # [16, 66]
        l2 = np.concatenate([-wi, wr], axis=1)
        v = l1.T @ k["cey"] + l2.T @ k["sey"]                    # [66, 128]
        o = k["icis"].T @ v + k["s_base"] * z                    # [128, 128]
        out[s] = o + k["c_total"]
    return out


def _shard_inputs(z, consts):
    ba, bb = _pack_blobs(consts)
    return [
        {"zc": np.ascontiguousarray(z[i * SPC:(i + 1) * SPC, 0]),
         "ba": ba, "bb": bb}
        for i in range(NCORES)
    ]


# ----------------------------------------------------------------------------
# Bass kernel
# ----------------------------------------------------------------------------

_NC_CACHE = {}
LAST_RESULT = None  # BassKernelResults of the most recent device run


def _build_nc():
    import concourse.bacc as bacc
    import concourse.mybir as mybir
    import concourse.tile as tile

    f32 = mybir.dt.float32
    ALU = mybir.AluOpType
    nc = bacc.Bacc("TRN2", target_bir_lowering=False, debug=False,
                   num_devices=NCORES)
    zc = nc.dram_tensor("zc", [SPC, D, D], f32, kind="ExternalInput")
    ba = nc.dram_tensor("ba", [D, AW], f32, kind="ExternalInput")
    bb = nc.dram_tensor("bb", [2 * NR, BW], f32, kind="ExternalInput")
    outc = nc.dram_tensor("outc", [SPC, D, D], f32, kind="ExternalOutput")

    W = 2 * NR  # 66

    with tile.TileContext(nc) as tc:
        with (
            tc.tile_pool(name="const", bufs=1) as cpool,
            tc.tile_pool(name="work", bufs=2) as wpool,
            tc.tile_pool(name="psum", bufs=1, space="PSUM") as ppool,
        ):
            # constants arrive as two packed blobs, issued from otherwise-idle
            # sequencers so the z DMA on sync isn't queued behind them
            t_ba = cpool.tile([D, AW], f32)
            t_bb = cpool.tile([W, BW], f32)
            nc.scalar.dma_start(t_ba[:], ba[:])
            nc.scalar.dma_start(t_bb[:], bb[:])

            t_z = wpool.tile([D, SPC, D], f32)
            nc.sync.dma_start(t_z[:], zc.rearrange("s x y -> x s y"))

            # x-forward DFT (output already transposed): PT_s = z_s.T @ FxC
            pt_ps = ppool.tile([D, SPC, W], f32)
            for s in range(SPC):
                nc.tensor.matmul(pt_ps[:, s, :], t_z[:, s, :],
                                 t_ba[:, A_FXC:A_FXC + W], start=True, stop=True)
            t_ptb = wpool.tile([D, SPC, W], f32)
            nc.vector.tensor_copy(t_ptb[:], pt_ps[:])

            # y-forward DFT, cos and sin blocks both on partitions 0:16
            # (two-input DVE ops require equal base partitions)
            qq_ps = ppool.tile([16, 2, SPC, W], f32)
            nc.tensor.matmul(qq_ps[:, 0, :, :], t_ba[:, A_CYC:A_CYC + 16],
                             t_ptb[:], start=True, stop=True)
            nc.tensor.matmul(qq_ps[:, 1, :, :], t_ba[:, A_CYS:A_CYS + 16],
                             t_ptb[:], start=True, stop=True)
            t_qq = wpool.tile([16, 2, SPC, W], f32)
            nc.vector.tensor_copy(t_qq[:], qq_ps[:])

            # complex assembly (transposed layout [ky, r]), batched over samples
            t_qt = wpool.tile([16, SPC, 2, NR], f32)
            nc.vector.tensor_add(t_qt[:, :, 0, :],
                                 t_qq[:, 0, :, 0:NR], t_qq[:, 1, :, NR:W])
            nc.vector.tensor_sub(t_qt[:, :, 1, :],
                                 t_qq[:, 0, :, NR:W], t_qq[:, 1, :, 0:NR])

            # spectral multiplier: W'r = Sr*Qr - Si*Qi ; W'i = Sr*Qi + Si*Qr
            srt = t_bb[0:16, B_SRT:B_SRT + W * SPC].rearrange(
                "k (s b r) -> k s b r", s=SPC, b=2)
            sit = t_bb[0:16, B_SIT:B_SIT + W * SPC].rearrange(
                "k (s b r) -> k s b r", s=SPC, b=2)
            t_m1 = wpool.tile([16, SPC, 2, NR], f32)
            t_m2 = wpool.tile([16, SPC, 2, NR], f32)
            nc.vector.tensor_mul(t_m1[:], srt, t_qt[:])
            nc.vector.tensor_mul(t_m2[:], sit, t_qt[:])
            t_l1 = wpool.tile([16, SPC, 2, NR], f32)
            t_l2 = wpool.tile([16, SPC, 2, NR], f32)
            nc.vector.tensor_sub(t_l1[:, :, 0, :], t_m1[:, :, 0, :], t_m2[:, :, 1, :])
            nc.vector.tensor_add(t_l1[:, :, 1, :], t_m1[:, :, 1, :], t_m2[:, :, 0, :])
            nc.vector.tensor_scalar_mul(t_l2[:, :, 0, :], t_l1[:, :, 1, :], -1.0)
            nc.vector.tensor_copy(t_l2[:, :, 1, :], t_l1[:, :, 0, :])

            # y-inverse: V = [Vr; Vi] = L1.T @ CEy + L2.T @ SEy
            v_ps = ppool.tile([W, SPC, D], f32)
            for s in range(SPC):
                nc.tensor.matmul(v_ps[:, s, :], t_l1[:, s, :, :],
                                 t_bb[0:16, B_CEY:B_CEY + D], start=True, stop=False)
                nc.tensor.matmul(v_ps[:, s, :], t_l2[:, s, :, :],
                                 t_bb[0:16, B_SEY:B_SEY + D], start=False, stop=True)
            t_v = wpool.tile([W, SPC, D], f32)
            nc.vector.tensor_copy(t_v[:], v_ps[:])

            # x-inverse + c_total (rank-1) accumulated in PSUM,
            # then out = s_base * z + O  in one fused DVE op
            o_ps = ppool.tile([D, SPC, D], f32)
            nc.tensor.matmul(o_ps[:, :, :], t_bb[0:1, B_CROW:B_CROW + D],
                             t_bb[0:1, B_ONES:B_ONES + SPC * D],
                             start=True, stop=False)
            for s in range(SPC):
                nc.tensor.matmul(o_ps[:, s, :], t_bb[:, B_ICIS:B_ICIS + D],
                                 t_v[:, s, :], start=False, stop=(s == SPC - 1))
            t_out = wpool.tile([D, SPC, D], f32)
            nc.vector.scalar_tensor_tensor(
                t_out[:], t_z[:], t_ba[:, A_SB:A_SB + 1], o_ps[:],
                op0=ALU.mult, op1=ALU.add)

            nc.sync.dma_start(outc.rearrange("s x y -> x s y"), t_out[:])

    nc.finalize()
    return nc


def _run_device(z, consts):
    global LAST_RESULT
    from concourse.bass_utils import run_bass_kernel_spmd

    if "nc" not in _NC_CACHE:
        _NC_CACHE["nc"] = _build_nc()
    nc = _NC_CACHE["nc"]
    in_maps = _shard_inputs(z, consts)
    res = run_bass_kernel_spmd(nc, in_maps, core_ids=list(range(NCORES)))
    LAST_RESULT = res
    out = np.empty((B, 1, D, D), np.float32)
    for i in range(NCORES):
        out[i * SPC:(i + 1) * SPC, 0] = res.results[i]["outc"]
    return out


def kernel(z, lift_w, lift_b, spec_w1, spec_w2, fl_bias, p1_w, p1_b, p2_w, p2_b,
           samp_ts):
    inputs = dict(z=z, lift_w=lift_w, lift_b=lift_b, spec_w1=spec_w1,
                  spec_w2=spec_w2, fl_bias=fl_bias, p1_w=p1_w, p1_b=p1_b,
                  p2_w=p2_w, p2_b=p2_b, samp_ts=samp_ts)
    consts = _fold_weights(inputs)
    z = np.asarray(z, np.float32)
    return _run_device(z, consts)


def kernel_numpy(z, **kw):
    """Pure-numpy path running the same folded math (validation only)."""
    inputs = dict(z=z, **kw)
    consts = _fold_weights(inputs)
    z = np.asarray(z, np.float32)
    out = np.empty((B, 1, D, D), np.float32)
    for i in range(NCORES):
        out[i * SPC:(i + 1) * SPC, 0] = _device_sim(z[i * SPC:(i + 1) * SPC, 0], consts)
    return out
